# revision 23
# baseline (speedup 1.0000x reference)
"""Trainium2 Bass kernel: 3x3 valid cross-correlation on a [4096, 8192] fp32 image.

Strategy: row-shard X across 8 NeuronCores (512 output rows each, 2-row
halo sliced host-side).  Per core the conv is computed as PSUM-accumulated
fp32r band matmuls: for each column shift dj, a banded stationary matrix
B_dj[q, p] = w[q-p, dj] contracts the row (partition) dimension while the
moving operand is the input tile column-shifted by dj.  fp32r streams at
bf16 speed on the PE (1 cycle/row for moving free dim >= 256) and is exact
for inputs pre-rounded to fp32r's 12-bit mantissa (done host-side; ~1e-4
relative rounding on the inputs, far inside tolerance).

Per-core layout:
 - 4 main row tiles of 126 output rows (input rows+2 <= 128 partitions),
   each loaded as two ~2 MB half-width DMAs; per tile, 4 output panels of
   2048 cols = 4 PSUM banks; 3 matmuls per 512-col chunk; one PSUM->SBUF
   copy (+bias) per panel alternating ScalarE/VectorE; one ~1 MB store per
   panel.
 - The ragged last 8 rows (504..511) are NOT a 5th full-cost tile (matmul
   time scales with columns, not rows).  Instead the 16 column chunks are
   packed into the partition dimension (chunk c, row p) -> partition
   8c + p with a block-diagonal banded stationary, reducing the ragged
   work from 48 to 9 matmuls.
Total per core: ~201 matmuls, ~33.6 MB HBM traffic (the memory roofline).
"""

import sys

for _p in ("/opt/trn_rl_repo", "/root/.axon_site/_ro/trn_rl_repo"):
    if _p not in sys.path:
        sys.path.append(_p)

from contextlib import ExitStack

import numpy as np

import concourse.bass as bass  # noqa: F401  (registers engine classes)
import concourse.tile as tile
from concourse import bacc, mybir
from concourse.bass_utils import run_bass_kernel_spmd

N_CORES = 8
H, W = 4096, 8192
KH, KW = 3, 3
OH, OW = H - KH + 1, W - KW + 1  # 4094 x 8190

ROWS_PER_CORE = 512           # output rows per core (core 7: 510 valid)
IN_ROWS = ROWS_PER_CORE + KH - 1  # 514 input rows per core
MTILE = 126                   # output rows per main row tile
FDIM = 512                    # PSUM chunk width (one bank of fp32)
PANEL = 4 * FDIM              # output cols per panel (4 PSUM banks)
HALF = 4096                   # half-width load granularity
N_MAIN = 4                    # main row tiles (4 x 126 = 504 rows)
RAG_R0 = N_MAIN * MTILE       # ragged rows 504..511
RAG_ROWS = ROWS_PER_CORE - RAG_R0          # 8
RAG_KIN = RAG_ROWS + KH - 1                # 10
# ragged chunk groups: (n_chunks, chunk0, F) packed into partitions
RAG_GROUPS = [(12, 0, FDIM), (3, 12, FDIM), (1, 15, OW - 15 * FDIM)]

_F32 = mybir.dt.float32
_F32R = mybir.dt.float32r


def _round_fp32r(x: np.ndarray) -> np.ndarray:
    """Round fp32 to fp32r's representable set (round-to-nearest-even at
    mantissa bit 12), matching neuron_dtypes.static_cast_fp32_to_fp32r."""
    b = np.ascontiguousarray(x).view(np.uint32).astype(np.uint64)
    lsb = (b >> 12) & 1
    r = (b + 0x7FF + lsb) & 0xFFFFF000
    return r.astype(np.uint32).view(np.float32).reshape(x.shape)


def _rag_band_cols():
    """Column layout of the ragged block-diagonal bands tensor."""
    segs = []
    col = 0
    for nch, _, _ in RAG_GROUPS:
        m = nch * RAG_ROWS
        segs.append((col, m))
        col += KW * m
    return segs, col


def _load_main_tile(nc, pools, aps, r0, split=2):
    """Issue the loads for a 126-row main tile, split into `split` (2 or 4)
    column sections.  Returns [(tile, panel_offset_in_tile), ...] per panel.
    Quarter splits let the first panel's compute start ~3 us earlier at a
    small DMA-efficiency cost, so they're used for the first tile only."""
    kin = MTILE + KH - 1
    sec_w = W // split
    pool = pools["x"] if split == 2 else pools["x1"]
    per_panel = []
    for h in range(split):
        wcols = min(sec_w + KW - 1, W - h * sec_w)
        xt = pool.tile([MTILE + 2, sec_w + KW - 1], _F32R,
                       tag=f"xt{split}_{h}")
        nc.sync.dma_start(xt[0:kin, 0:wcols],
                          aps["x"][r0:r0 + kin, h * sec_w:h * sec_w + wcols])
        for p in range(4 // split):
            per_panel.append((xt, p * PANEL))
    return per_panel


def _emit_main_tile(nc, pools, aps, r0, per_panel, split_stores=False):
    """Compute one 126-row main tile: 4 panels x (12 matmuls, 2 half-panel
    copies, 1 store).  split_stores issues one store per half-panel instead
    (shorter kernel tail; used for the last tile)."""
    opool, pspool = pools["o"], pools["ps"]
    y_out, bands, bias_t = aps["y"], aps["bands"], aps["bias"]
    rows = MTILE
    kin = rows + KH - 1

    for p in range(4):
        c0 = p * PANEL
        cols = min(PANEL, OW - c0)
        xt, off = per_panel[p]
        ot = opool.tile([MTILE, PANEL], _F32)
        # Two half-panel PSUM tiles (2 banks each) for finer PE->copy->store
        # pipelining; copies alternate ScalarE/VectorE.
        for hp in range(2):
            ps = pspool.tile([128, PANEL // 2], _F32, tag="ps")
            g0 = hp * (PANEL // 2)
            pcols = min(PANEL // 2, cols - g0)
            for ci in range(2):
                f0 = ci * FDIM
                f1 = min(f0 + FDIM, pcols)
                if f1 <= f0:
                    continue
                for dj in range(KW):
                    nc.tensor.matmul(
                        ps[0:rows, f0:f1],
                        bands[0:kin, dj * MTILE:dj * MTILE + rows],
                        xt[0:kin, off + g0 + f0 + dj:off + g0 + f1 + dj],
                        start=(dj == 0),
                        stop=(dj == KW - 1),
                    )
            if pools["parity"] == 0:
                nc.scalar.add(ot[0:rows, g0:g0 + pcols], ps[0:rows, 0:pcols],
                              bias_t[0:rows, 0:1])
            else:
                nc.vector.tensor_scalar_add(ot[0:rows, g0:g0 + pcols],
                                            ps[0:rows, 0:pcols],
                                            bias_t[0:rows, 0:1])
            pools["parity"] ^= 1
            if split_stores:
                # End-of-kernel stores go on the (by now idle) SP HWDGE ring
                # so the GpSimd tail drain isn't the last thing standing.
                nc.sync.dma_start(
                    y_out[r0:r0 + rows, c0 + g0:c0 + g0 + pcols],
                    ot[0:rows, g0:g0 + pcols])
        if not split_stores:
            nc.gpsimd.dma_start(y_out[r0:r0 + rows, c0:c0 + cols],
                                ot[0:rows, 0:cols])


def _emit_ragged(nc, pools, aps):
    """Ragged rows 504..511: chunks packed into partitions, 3 groups x 3
    matmuls with block-diagonal bands."""
    rpool, pspool = pools["r"], pools["ps"]
    x_in, y_out, rbands, bias_t = aps["x"], aps["y"], aps["rbands"], aps["bias"]
    segs, _ = _rag_band_cols()

    for gi, (nch, ch0, fdim) in enumerate(RAG_GROUPS):
        k = nch * RAG_KIN
        m = nch * RAG_ROWS
        col0, _ = segs[gi]
        xts = []
        for dj in range(KW):
            xt = rpool.tile([k, fdim], _F32R, tag=f"rx{gi}_{dj}")
            src = x_in[RAG_R0:RAG_R0 + RAG_KIN,
                       ch0 * FDIM + dj:ch0 * FDIM + dj + nch * fdim]
            if nch > 1:
                src = src.rearrange("q (c t) -> c q t", t=fdim)
            # Keep the 9 small ragged loads off the main load FIFO: issue
            # them on the otherwise-idle ACT HWDGE ring.
            nc.scalar.dma_start(xt[:], src)
            xts.append(xt)
        ps = pspool.tile([128, PANEL // 2], _F32, tag="ps")
        for dj in range(KW):
            nc.tensor.matmul(ps[0:m, 0:fdim],
                             rbands[0:k, col0 + dj * m:col0 + (dj + 1) * m],
                             xts[dj][:, :],
                             start=(dj == 0), stop=(dj == KW - 1))
        ot = rpool.tile([m, fdim], _F32, tag=f"ro{gi}")
        if pools["parity"] == 0:
            nc.scalar.add(ot[:, :], ps[0:m, 0:fdim], bias_t[0:m, 0:1])
        else:
            nc.vector.tensor_scalar_add(ot[:, :], ps[0:m, 0:fdim],
                                        bias_t[0:m, 0:1])
        pools["parity"] ^= 1
        dst = y_out[RAG_R0:RAG_R0 + RAG_ROWS,
                    ch0 * FDIM:ch0 * FDIM + nch * fdim]
        if nch > 1:
            dst = dst.rearrange("p (c t) -> c p t", t=fdim)
        nc.gpsimd.dma_start(dst, ot[:, :])


def _build_program():
    nc = bacc.Bacc("TRN2", target_bir_lowering=False, debug=False,
                   num_devices=N_CORES)
    _, rb_cols = _rag_band_cols()
    x_in = nc.dram_tensor("x", [IN_ROWS, W], _F32R, kind="ExternalInput").ap()
    bands_in = nc.dram_tensor("bands", [MTILE + 2, KW * MTILE], _F32R,
                              kind="ExternalInput").ap()
    rbands_in = nc.dram_tensor("rbands", [RAG_GROUPS[0][0] * RAG_KIN, rb_cols],
                               _F32R, kind="ExternalInput").ap()
    bias_in = nc.dram_tensor("bias", [128, 1], _F32, kind="ExternalInput").ap()
    y_out = nc.dram_tensor("y", [ROWS_PER_CORE, OW], _F32,
                           kind="ExternalOutput").ap()

    with tile.TileContext(nc) as tc, ExitStack() as ctx:
        const_pool = ctx.enter_context(tc.tile_pool(name="const", bufs=1))
        xpool = ctx.enter_context(tc.tile_pool(name="xt", bufs=3))
        xpool1 = ctx.enter_context(tc.tile_pool(name="xt1", bufs=1))
        opool = ctx.enter_context(tc.tile_pool(name="ot", bufs=4))
        rpool = ctx.enter_context(tc.tile_pool(name="rt", bufs=1))
        pspool = ctx.enter_context(tc.tile_pool(name="ps", bufs=4, space="PSUM"))

        pools = {"x": xpool, "x1": xpool1, "o": opool, "r": rpool,
                 "ps": pspool, "parity": 0}
        aps = {"x": x_in, "y": y_out, "bias": None}

        # First tile's data loads go out before the (tiny) constant loads so
        # the PE's first matmul isn't gated on a DMA queue warmup tail.
        h0 = _load_main_tile(nc, pools, aps, 0)

        bands = const_pool.tile([MTILE + 2, KW * MTILE], _F32R)
        nc.sync.dma_start(bands[:], bands_in[:, :])
        rbands = const_pool.tile([RAG_GROUPS[0][0] * RAG_KIN, rb_cols], _F32R)
        nc.sync.dma_start(rbands[:], rbands_in[:, :])
        bias_t = const_pool.tile([128, 1], _F32)
        nc.sync.dma_start(bias_t[:], bias_in[:, :])
        aps.update({"bands": bands, "rbands": rbands, "bias": bias_t})

        h1 = _load_main_tile(nc, pools, aps, MTILE)
        _emit_main_tile(nc, pools, aps, 0, h0)
        h2 = _load_main_tile(nc, pools, aps, 2 * MTILE)
        _emit_main_tile(nc, pools, aps, MTILE, h1)
        _emit_ragged(nc, pools, aps)
        h3 = _load_main_tile(nc, pools, aps, 3 * MTILE)
        _emit_main_tile(nc, pools, aps, 2 * MTILE, h2)
        _emit_main_tile(nc, pools, aps, 3 * MTILE, h3, split_stores=True)

    nc.compile()
    return nc


_NC_CACHE = None


def _get_program():
    global _NC_CACHE
    if _NC_CACHE is None:
        _NC_CACHE = _build_program()
    return _NC_CACHE


def _make_bands(weight: np.ndarray) -> np.ndarray:
    """bands[q, dj*MTILE + p] = weight[q - p, dj] for q - p in [0, KH)."""
    k = MTILE + 2
    bands = np.zeros((k, KW * MTILE), dtype=np.float32)
    idx = np.arange(MTILE)
    for dj in range(KW):
        for di in range(KH):
            bands[idx + di, dj * MTILE + idx] = weight[di, dj]
    return bands


def _make_rbands(weight: np.ndarray) -> np.ndarray:
    """Block-diagonal ragged bands: within group g, block c maps input
    partitions RAG_KIN*c + q to output partitions RAG_ROWS*c + p with
    weight[q - p, dj]."""
    segs, rb_cols = _rag_band_cols()
    k0 = RAG_GROUPS[0][0] * RAG_KIN
    rb = np.zeros((k0, rb_cols), dtype=np.float32)
    idx = np.arange(RAG_ROWS)
    for gi, (nch, _, _) in enumerate(RAG_GROUPS):
        col0, m = segs[gi]
        for dj in range(KW):
            for c in range(nch):
                for di in range(KH):
                    rb[RAG_KIN * c + idx + di,
                       col0 + dj * m + RAG_ROWS * c + idx] = weight[di, dj]
    return rb


def _shard_inputs(X: np.ndarray, weight: np.ndarray, bias: np.ndarray):
    Xr = _round_fp32r(X.astype(np.float32, copy=False))
    w = weight.astype(np.float32, copy=False)
    bands = _round_fp32r(_make_bands(w))
    rbands = _round_fp32r(_make_rbands(w))
    bias_col = np.ascontiguousarray(
        np.broadcast_to(bias.astype(np.float32, copy=False).reshape(1, 1),
                        (128, 1)))
    in_maps = []
    for c in range(N_CORES):
        r0 = c * ROWS_PER_CORE
        r1 = min(r0 + IN_ROWS, H)
        xs = Xr[r0:r1]
        if xs.shape[0] < IN_ROWS:  # pad last core's halo with zeros
            pad = np.zeros((IN_ROWS - xs.shape[0], W), dtype=np.float32)
            xs = np.concatenate([xs, pad], axis=0)
        in_maps.append({
            "x": np.ascontiguousarray(xs),
            "bands": bands,
            "rbands": rbands,
            "bias": bias_col,
        })
    return in_maps


def _gather(results) -> np.ndarray:
    out = np.empty((OH, OW), dtype=np.float32)
    for c in range(N_CORES):
        r0 = c * ROWS_PER_CORE
        rows = min(ROWS_PER_CORE, OH - r0)
        out[r0:r0 + rows] = results[c]["y"][0:rows]
    return out


def run(X, weight, bias, trace=False, **spmd_kwargs):
    """Full-input entry point; returns (output, BassKernelResults)."""
    nc = _get_program()
    in_maps = _shard_inputs(X, weight, bias)
    res = run_bass_kernel_spmd(nc, in_maps, list(range(N_CORES)),
                               trace=trace, **spmd_kwargs)
    return _gather(res.results), res


def kernel(X, weight, bias):
    out, _ = run(X, weight, bias)
    return out


# revision 24
# speedup vs baseline: 1.0919x; 1.0919x over previous
"""Trainium2 Bass kernel: 3x3 valid cross-correlation on a [4096, 8192] fp32 image.

Strategy: row-shard X across 8 NeuronCores (512 output rows each, 2-row
halo sliced host-side).  Per core the conv is computed as PSUM-accumulated
fp32r band matmuls: for each column shift dj, a banded stationary matrix
B_dj[q, p] = w[q-p, dj] contracts the row (partition) dimension while the
moving operand is the input tile column-shifted by dj.  fp32r streams at
bf16 speed on the PE (1 cycle/row for moving free dim >= 256) and is exact
for inputs pre-rounded to fp32r's 12-bit mantissa (done host-side; ~1e-4
relative rounding on the inputs, far inside tolerance).

Per-core layout:
 - 4 main row tiles of 126 output rows (input rows+2 <= 128 partitions),
   each loaded as two ~2 MB half-width DMAs; per tile, 4 output panels of
   2048 cols = 4 PSUM banks; 3 matmuls per 512-col chunk; one PSUM->SBUF
   copy (+bias) per panel alternating ScalarE/VectorE; one ~1 MB store per
   panel.
 - The ragged last 8 rows (504..511) are NOT a 5th full-cost tile (matmul
   time scales with columns, not rows).  Instead the 16 column chunks are
   packed into the partition dimension (chunk c, row p) -> partition
   8c + p with a block-diagonal banded stationary, reducing the ragged
   work from 48 to 9 matmuls.
Total per core: ~201 matmuls, ~33.6 MB HBM traffic (the memory roofline).
"""

import sys

for _p in ("/opt/trn_rl_repo", "/root/.axon_site/_ro/trn_rl_repo"):
    if _p not in sys.path:
        sys.path.append(_p)

from contextlib import ExitStack

import numpy as np

import concourse.bass as bass  # noqa: F401  (registers engine classes)
import concourse.tile as tile
from concourse import bacc, mybir
from concourse.bass_utils import run_bass_kernel_spmd

N_CORES = 8
H, W = 4096, 8192
KH, KW = 3, 3
OH, OW = H - KH + 1, W - KW + 1  # 4094 x 8190

ROWS_PER_CORE = 512           # output rows per core (core 7: 510 valid)
IN_ROWS = ROWS_PER_CORE + KH - 1  # 514 input rows per core
MTILE = 126                   # output rows per main row tile
FDIM = 512                    # PSUM chunk width (one bank of fp32)
PANEL = 4 * FDIM              # output cols per panel (4 PSUM banks)
HALF = 4096                   # half-width load granularity
N_MAIN = 4                    # main row tiles (4 x 126 = 504 rows)
RAG_R0 = N_MAIN * MTILE       # ragged rows 504..511
RAG_ROWS = ROWS_PER_CORE - RAG_R0          # 8
RAG_KIN = RAG_ROWS + KH - 1                # 10
# ragged chunk groups: (n_chunks, chunk0, F) packed into partitions
RAG_GROUPS = [(12, 0, FDIM), (3, 12, FDIM), (1, 15, OW - 15 * FDIM)]

_F32 = mybir.dt.float32
_F32R = mybir.dt.float32r


def _round_fp32r(x: np.ndarray) -> np.ndarray:
    """Round fp32 to fp32r's representable set (round-to-nearest-even at
    mantissa bit 12), matching neuron_dtypes.static_cast_fp32_to_fp32r."""
    b = np.ascontiguousarray(x).view(np.uint32).astype(np.uint64)
    lsb = (b >> 12) & 1
    r = (b + 0x7FF + lsb) & 0xFFFFF000
    return r.astype(np.uint32).view(np.float32).reshape(x.shape)


def _rag_band_cols():
    """Column layout of the ragged block-diagonal bands tensor."""
    segs = []
    col = 0
    for nch, _, _ in RAG_GROUPS:
        m = nch * RAG_ROWS
        segs.append((col, m))
        col += KW * m
    return segs, col


def _load_main_tile(nc, pools, aps, r0, split=2):
    """Issue the loads for a 126-row main tile, split into `split` (2 or 4)
    column sections.  Returns [(tile, panel_offset_in_tile), ...] per panel.
    Quarter splits let the first panel's compute start ~3 us earlier at a
    small DMA-efficiency cost, so they're used for the first tile only."""
    kin = MTILE + KH - 1
    sec_w = W // split
    pool = pools["x"] if split == 2 else pools["x1"]
    per_panel = []
    for h in range(split):
        wcols = min(sec_w + KW - 1, W - h * sec_w)
        xt = pool.tile([MTILE + 2, sec_w + KW - 1], _F32R,
                       tag=f"xt{split}_{h}")
        nc.sync.dma_start(xt[0:kin, 0:wcols],
                          aps["x"][r0:r0 + kin, h * sec_w:h * sec_w + wcols])
        for p in range(4 // split):
            per_panel.append((xt, p * PANEL))
    return per_panel


def _emit_main_tile(nc, pools, aps, r0, per_panel, split_stores=False):
    """Compute one 126-row main tile: 4 panels x (12 matmuls, 2 half-panel
    copies, 1 store).  split_stores issues one store per half-panel instead
    (shorter kernel tail; used for the last tile)."""
    opool, pspool = pools["o"], pools["ps"]
    y_out, bands, bias_t = aps["y"], aps["bands"], aps["bias"]
    rows = MTILE
    kin = rows + KH - 1

    for p in range(4):
        c0 = p * PANEL
        cols = min(PANEL, OW - c0)
        xt, off = per_panel[p]
        ot = opool.tile([MTILE, PANEL], _F32)
        # Two half-panel PSUM tiles (2 banks each) for finer PE->copy->store
        # pipelining; copies alternate ScalarE/VectorE.
        for hp in range(2):
            ps = pspool.tile([128, PANEL // 2], _F32, tag="ps")
            g0 = hp * (PANEL // 2)
            pcols = min(PANEL // 2, cols - g0)
            for ci in range(2):
                f0 = ci * FDIM
                f1 = min(f0 + FDIM, pcols)
                if f1 <= f0:
                    continue
                for dj in range(KW):
                    nc.tensor.matmul(
                        ps[0:rows, f0:f1],
                        bands[0:kin, dj * MTILE:dj * MTILE + rows],
                        xt[0:kin, off + g0 + f0 + dj:off + g0 + f1 + dj],
                        start=(dj == 0),
                        stop=(dj == KW - 1),
                    )
            if pools["parity"] == 0:
                nc.scalar.add(ot[0:rows, g0:g0 + pcols], ps[0:rows, 0:pcols],
                              bias_t[0:rows, 0:1])
            else:
                nc.vector.tensor_scalar_add(ot[0:rows, g0:g0 + pcols],
                                            ps[0:rows, 0:pcols],
                                            bias_t[0:rows, 0:1])
            pools["parity"] ^= 1
            if split_stores:
                nc.gpsimd.dma_start(
                    y_out[r0:r0 + rows, c0 + g0:c0 + g0 + pcols],
                    ot[0:rows, g0:g0 + pcols])
        if not split_stores:
            nc.gpsimd.dma_start(y_out[r0:r0 + rows, c0:c0 + cols],
                                ot[0:rows, 0:cols])


def _emit_ragged(nc, pools, aps):
    """Ragged rows 504..511: chunks packed into partitions, 3 groups x 3
    matmuls with block-diagonal bands."""
    rpool, pspool = pools["r"], pools["ps"]
    x_in, y_out, rbands, bias_t = aps["x"], aps["y"], aps["rbands"], aps["bias"]
    segs, _ = _rag_band_cols()

    for gi, (nch, ch0, fdim) in enumerate(RAG_GROUPS):
        k = nch * RAG_KIN
        m = nch * RAG_ROWS
        col0, _ = segs[gi]
        xts = []
        for dj in range(KW):
            xt = rpool.tile([k, fdim], _F32R, tag=f"rx{gi}_{dj}")
            src = x_in[RAG_R0:RAG_R0 + RAG_KIN,
                       ch0 * FDIM + dj:ch0 * FDIM + dj + nch * fdim]
            if nch > 1:
                src = src.rearrange("q (c t) -> c q t", t=fdim)
            nc.sync.dma_start(xt[:], src)
            xts.append(xt)
        ps = pspool.tile([128, PANEL // 2], _F32, tag="ps")
        for dj in range(KW):
            nc.tensor.matmul(ps[0:m, 0:fdim],
                             rbands[0:k, col0 + dj * m:col0 + (dj + 1) * m],
                             xts[dj][:, :],
                             start=(dj == 0), stop=(dj == KW - 1))
        ot = rpool.tile([m, fdim], _F32, tag=f"ro{gi}")
        if pools["parity"] == 0:
            nc.scalar.add(ot[:, :], ps[0:m, 0:fdim], bias_t[0:m, 0:1])
        else:
            nc.vector.tensor_scalar_add(ot[:, :], ps[0:m, 0:fdim],
                                        bias_t[0:m, 0:1])
        pools["parity"] ^= 1
        dst = y_out[RAG_R0:RAG_R0 + RAG_ROWS,
                    ch0 * FDIM:ch0 * FDIM + nch * fdim]
        if nch > 1:
            dst = dst.rearrange("p (c t) -> c p t", t=fdim)
        nc.gpsimd.dma_start(dst, ot[:, :])


def _build_program():
    nc = bacc.Bacc("TRN2", target_bir_lowering=False, debug=False,
                   num_devices=N_CORES)
    _, rb_cols = _rag_band_cols()
    x_in = nc.dram_tensor("x", [IN_ROWS, W], _F32R, kind="ExternalInput").ap()
    bands_in = nc.dram_tensor("bands", [MTILE + 2, KW * MTILE], _F32R,
                              kind="ExternalInput").ap()
    rbands_in = nc.dram_tensor("rbands", [RAG_GROUPS[0][0] * RAG_KIN, rb_cols],
                               _F32R, kind="ExternalInput").ap()
    bias_in = nc.dram_tensor("bias", [128, 1], _F32, kind="ExternalInput").ap()
    y_out = nc.dram_tensor("y", [ROWS_PER_CORE, OW], _F32,
                           kind="ExternalOutput").ap()

    with tile.TileContext(nc) as tc, ExitStack() as ctx:
        const_pool = ctx.enter_context(tc.tile_pool(name="const", bufs=1))
        xpool = ctx.enter_context(tc.tile_pool(name="xt", bufs=3))
        xpool1 = ctx.enter_context(tc.tile_pool(name="xt1", bufs=1))
        opool = ctx.enter_context(tc.tile_pool(name="ot", bufs=4))
        rpool = ctx.enter_context(tc.tile_pool(name="rt", bufs=1))
        pspool = ctx.enter_context(tc.tile_pool(name="ps", bufs=4, space="PSUM"))

        pools = {"x": xpool, "x1": xpool1, "o": opool, "r": rpool,
                 "ps": pspool, "parity": 0}
        aps = {"x": x_in, "y": y_out, "bias": None}

        # First tile's data loads go out before the (tiny) constant loads so
        # the PE's first matmul isn't gated on a DMA queue warmup tail.
        h0 = _load_main_tile(nc, pools, aps, 0)

        bands = const_pool.tile([MTILE + 2, KW * MTILE], _F32R)
        nc.sync.dma_start(bands[:], bands_in[:, :])
        rbands = const_pool.tile([RAG_GROUPS[0][0] * RAG_KIN, rb_cols], _F32R)
        nc.sync.dma_start(rbands[:], rbands_in[:, :])
        bias_t = const_pool.tile([128, 1], _F32)
        nc.sync.dma_start(bias_t[:], bias_in[:, :])
        aps.update({"bands": bands, "rbands": rbands, "bias": bias_t})

        h1 = _load_main_tile(nc, pools, aps, MTILE)
        _emit_main_tile(nc, pools, aps, 0, h0)
        h2 = _load_main_tile(nc, pools, aps, 2 * MTILE)
        _emit_main_tile(nc, pools, aps, MTILE, h1)
        _emit_ragged(nc, pools, aps)
        h3 = _load_main_tile(nc, pools, aps, 3 * MTILE)
        _emit_main_tile(nc, pools, aps, 2 * MTILE, h2)
        _emit_main_tile(nc, pools, aps, 3 * MTILE, h3)

    nc.compile()
    return nc


_NC_CACHE = None


def _get_program():
    global _NC_CACHE
    if _NC_CACHE is None:
        _NC_CACHE = _build_program()
    return _NC_CACHE


def _make_bands(weight: np.ndarray) -> np.ndarray:
    """bands[q, dj*MTILE + p] = weight[q - p, dj] for q - p in [0, KH)."""
    k = MTILE + 2
    bands = np.zeros((k, KW * MTILE), dtype=np.float32)
    idx = np.arange(MTILE)
    for dj in range(KW):
        for di in range(KH):
            bands[idx + di, dj * MTILE + idx] = weight[di, dj]
    return bands


def _make_rbands(weight: np.ndarray) -> np.ndarray:
    """Block-diagonal ragged bands: within group g, block c maps input
    partitions RAG_KIN*c + q to output partitions RAG_ROWS*c + p with
    weight[q - p, dj]."""
    segs, rb_cols = _rag_band_cols()
    k0 = RAG_GROUPS[0][0] * RAG_KIN
    rb = np.zeros((k0, rb_cols), dtype=np.float32)
    idx = np.arange(RAG_ROWS)
    for gi, (nch, _, _) in enumerate(RAG_GROUPS):
        col0, m = segs[gi]
        for dj in range(KW):
            for c in range(nch):
                for di in range(KH):
                    rb[RAG_KIN * c + idx + di,
                       col0 + dj * m + RAG_ROWS * c + idx] = weight[di, dj]
    return rb


def _shard_inputs(X: np.ndarray, weight: np.ndarray, bias: np.ndarray):
    Xr = _round_fp32r(X.astype(np.float32, copy=False))
    w = weight.astype(np.float32, copy=False)
    bands = _round_fp32r(_make_bands(w))
    rbands = _round_fp32r(_make_rbands(w))
    bias_col = np.ascontiguousarray(
        np.broadcast_to(bias.astype(np.float32, copy=False).reshape(1, 1),
                        (128, 1)))
    in_maps = []
    for c in range(N_CORES):
        r0 = c * ROWS_PER_CORE
        r1 = min(r0 + IN_ROWS, H)
        xs = Xr[r0:r1]
        if xs.shape[0] < IN_ROWS:  # pad last core's halo with zeros
            pad = np.zeros((IN_ROWS - xs.shape[0], W), dtype=np.float32)
            xs = np.concatenate([xs, pad], axis=0)
        in_maps.append({
            "x": np.ascontiguousarray(xs),
            "bands": bands,
            "rbands": rbands,
            "bias": bias_col,
        })
    return in_maps


def _gather(results) -> np.ndarray:
    out = np.empty((OH, OW), dtype=np.float32)
    for c in range(N_CORES):
        r0 = c * ROWS_PER_CORE
        rows = min(ROWS_PER_CORE, OH - r0)
        out[r0:r0 + rows] = results[c]["y"][0:rows]
    return out


def run(X, weight, bias, trace=False, **spmd_kwargs):
    """Full-input entry point; returns (output, BassKernelResults)."""
    nc = _get_program()
    in_maps = _shard_inputs(X, weight, bias)
    res = run_bass_kernel_spmd(nc, in_maps, list(range(N_CORES)),
                               trace=trace, **spmd_kwargs)
    return _gather(res.results), res


def kernel(X, weight, bias):
    out, _ = run(X, weight, bias)
    return out
